# revision 12
# baseline (speedup 1.0000x reference)
"""GAT layer (segment-softmax message passing) on 8 Trainium2 NeuronCores.

Strategy (per core c of NC=8, SPMD single program, per-core input maps):
  - Nodes sharded by destination: core c owns dst rows [c*NPC, (c+1)*NPC).
  - Each core computes the full z = h @ W^T itself (hT pre-rolled so its own
    nodes are rows [0, NPC)), storing z twice in DRAM:
      z_all : partition-major pseudo-row layout (gather source for z_src)
      z_own : row-major first ZOWN_ROWS rows (gather source for z_dst)
  - Edges are bucketed by (dst block of 128 nodes, src int16-gather window).
    Blocks are grouped into supers of 8. Slot layout per super: for each of
    the 4 windows, the 8 blocks' chunks concatenated (each chunk padded to
    x128, capacity = max over cores, so the layout is core-invariant).
  - Per (super, window) region: one chunked dma_gather of z_src from the
    window of z_all and one of z_dst from z_own; DVE dot + leaky-relu;
    ScalarE exp (no max subtraction: softmax is shift-invariant and fp32
    holds exp(~45)); vals = [ex * z_src, ex] in bf16; a one-hot S matrix
    [128 edges x 128 slots] built on DVE from host slotids (is_equal vs an
    iota row; padding slots get slotid=-1 so their column is all-zero).
  - Aggregation: per 128-edge sub-tile, TensorE matmul S^T @ vals
    accumulates into the dst block's PSUM tile [128, 65] (fp32 accumulate,
    start/stop over the block's subs). No scatter-add, no dst-row
    duplication concerns, no virtual nodes.
  - After a super's 4 regions: PSUM -> SBUF, normalize by the ex column,
    ELU, and one contiguous DMA write of the 8 blocks' output rows.

The host does only sharding/layout work: bucketing, padding, int16/bf16
index tensors, and the h^T roll. All FLOPs happen on device.
"""

import os
import sys

sys.path.insert(0, "/opt/trn_rl_repo")

import ml_dtypes
import numpy as np

import concourse.bacc as bacc
import concourse.mybir as mybir
import concourse.tile as tile
from concourse.bass_utils import run_bass_kernel_spmd

F32 = mybir.dt.float32
BF16 = mybir.dt.bfloat16
I16 = mybir.dt.int16
AF = mybir.ActivationFunctionType
ALU = mybir.AluOpType
AX = mybir.AxisListType

LAST_RESULTS = None  # test harness reads exec_time_ns from here

N = 100000
IN_DIM = 128
OUT_DIM = 64
NC = 8
NPC = N // NC  # 12500
SLOPE = 0.2

NT_G = (N + 127) // 128  # 782 GEMM row tiles
NROWS = NT_G * 128  # 100096 padded node rows
NWIN = 4
CHUNK_PSEUDO = NROWS // NWIN  # 25024 pseudo rows per int16 gather window
ZOWN_ROWS = ((NPC + 127) // 128) * 128  # 12544
NBLK = ZOWN_ROWS // 128  # 98 dst blocks per core
SUPER = 4  # dst blocks per super-tile
NSUP = (NBLK + SUPER - 1) // SUPER  # 13
GMAX = 8192  # max idxs per SWDGE gather call
VW = 65  # vals width: 64 features + ex column


def _wrap16(a):
    """[n] int -> [128, n//16] int16 (element i at [i%16, i//16], tiled x8)."""
    w = a.astype(np.int16).reshape(-1, 16).T.copy()
    return np.tile(w, (8, 1))


def _tile128(a, dtype):
    """[n] -> [128, n//128] (element j at [j%128, j//128])."""
    return np.ascontiguousarray(a.reshape(-1, 128).T).astype(dtype)


def _plan(h, W, src, dst):
    h = np.asarray(h, np.float32)
    W = np.asarray(W, np.float32)
    src = np.asarray(src).astype(np.int64)
    dst = np.asarray(dst).astype(np.int64)

    # ---- per-core edge bucketing by (dst block, src window) -------------
    per_core = []
    counts = np.zeros((NC, NBLK, NWIN), np.int64)
    for c in range(NC):
        sel = np.flatnonzero(dst // NPC == c)
        d_l = (dst[sel] - c * NPC).astype(np.int32)
        s_r = ((src[sel] - c * NPC) % N).astype(np.int64)
        pseudo = ((s_r % 128) * NT_G + s_r // 128).astype(np.int32)
        w = pseudo // CHUNK_PSEUDO
        blk = d_l >> 7
        key = (blk.astype(np.int64) * NWIN + w) * E_KEY + pseudo
        order = np.argsort(key, kind="stable")
        d_l, pseudo, w, blk = d_l[order], pseudo[order], w[order], blk[order]
        np.add.at(counts[c], (blk, w), 1)
        per_core.append((d_l, pseudo, w, blk))

    C = counts.max(axis=0)  # [NBLK, NWIN]
    C = ((C + 127) // 128) * 128
    C[:, 0] = np.maximum(C[:, 0], 128)  # every block gets >=1 sub

    # ---- static slot layout (identical across cores) --------------------
    chunk_off = np.zeros((NBLK, NWIN), np.int64)
    region = []  # per super: list over w of (region_off, region_len)
    off = 0
    for s in range(NSUP):
        blks = range(s * SUPER, min((s + 1) * SUPER, NBLK))
        regs = []
        for w in range(NWIN):
            r0 = off
            for b in blks:
                chunk_off[b, w] = off
                off += C[b, w]
            regs.append((r0, off - r0))
        region.append(regs)
    TOT = off
    RCMAX = max(l for regs in region for _, l in regs)

    # ---- per-core index tensors -----------------------------------------
    hT = np.ascontiguousarray(h.T)
    wT = np.ascontiguousarray(W.T)
    iota = np.broadcast_to(np.arange(128, dtype=np.float32), (128, 128))
    in_maps = []
    for c in range(NC):
        d_l, pseudo, w, blk = per_core[c]
        gsrc = np.zeros(TOT, np.int32)
        gdst = np.zeros(TOT, np.int32)
        slotid = np.full(TOT, -1.0, np.float32)
        # edges are sorted by (blk, w, pseudo); chunks are contiguous runs
        cnt = counts[c]
        pos = 0
        for b in range(NBLK):
            for ww in range(NWIN):
                k = cnt[b, ww]
                if k == 0:
                    continue
                o = chunk_off[b, ww]
                sl = slice(pos, pos + k)
                gsrc[o:o + k] = pseudo[sl] - ww * CHUNK_PSEUDO
                gdst[o:o + k] = d_l[sl]
                slotid[o:o + k] = (d_l[sl] & 127).astype(np.float32)
                pos += k
        assert pos == len(d_l)
        hp = np.zeros((IN_DIM, NROWS), np.float32)
        hp[:, :N] = np.roll(hT, -c * NPC, axis=1)
        in_maps.append({
            "hT": hp,
            "wT": wT,
            "iota": iota.astype(ml_dtypes.bfloat16),
            "gsrc_idx": _wrap16(gsrc),
            "gdst_idx": _wrap16(gdst),
            "slotid": _tile128(slotid, ml_dtypes.bfloat16),
        })
    return in_maps, C, chunk_off, region, TOT, RCMAX


E_KEY = 1 << 18  # > CHUNK_PSEUDO, for host-side lexsort key


def _build(h, W, src, dst):
    in_maps, C, chunk_off, region, TOT, RCMAX = _plan(h, W, src, dst)
    RCCOL = RCMAX // 128

    nc = bacc.Bacc(None, target_bir_lowering=False, debug=False)
    hT_d = nc.declare_dram_parameter("hT", [IN_DIM, NROWS], F32, isOutput=False)
    wT_d = nc.declare_dram_parameter("wT", [IN_DIM, OUT_DIM], F32, isOutput=False)
    iota_d = nc.declare_dram_parameter("iota", [128, 128], BF16, isOutput=False)
    gsrc_d = nc.declare_dram_parameter("gsrc_idx", [128, TOT // 16], I16, isOutput=False)
    gdst_d = nc.declare_dram_parameter("gdst_idx", [128, TOT // 16], I16, isOutput=False)
    slot_d = nc.declare_dram_parameter("slotid", [128, TOT // 128], BF16, isOutput=False)
    out_d = nc.declare_dram_parameter("out", [ZOWN_ROWS, OUT_DIM], F32, isOutput=True)
    DEBUG = bool(int(os.environ.get("GAT_DEBUG", "0")))
    if DEBUG:
        astg_d = nc.declare_dram_parameter("stg_dump", [NSUP, 128, SUPER, VW], F32, isOutput=True)
        e_d = nc.declare_dram_parameter("e_dump", [128, TOT // 128], F32, isOutput=True)
        s_d = nc.declare_dram_parameter("s_dump", [128, RCMAX // 128, 128], F32, isOutput=True)

    z_all = nc.dram_tensor("z_all", [NROWS, OUT_DIM], F32)  # pseudo rows
    z_own = nc.dram_tensor("z_own", [ZOWN_ROWS, OUT_DIM], F32)

    QB = 8
    with tile.TileContext(nc) as tc:
        with tc.tile_pool(name="w", bufs=1) as wpool, \
             tc.tile_pool(name="gps", bufs=4, space="PSUM") as gpspool, \
             tc.tile_pool(name="agg", bufs=1, space="PSUM") as apool, \
             tc.tile_pool(name="hst", bufs=3) as hpool, \
             tc.tile_pool(name="zst", bufs=3) as zpool, \
             tc.tile_pool(name="gat", bufs=2) as gpool, \
             tc.tile_pool(name="sm", bufs=2) as spool, \
             tc.tile_pool(name="fin", bufs=2) as fpool:

            # ---------------- phase A: z = h @ W^T ----------------------
            wt = wpool.tile([IN_DIM, OUT_DIM], F32)
            nc.sync.dma_start(wt[:], wT_d[:])
            it_t = wpool.tile([128, 128], BF16)
            nc.sync.dma_start(it_t[:], iota_d[:])
            z_all3 = z_all[:].rearrange("(p i) d -> p i d", p=128)
            for i0 in range(0, NT_G, QB):
                qb = min(QB, NT_G - i0)
                hstage = hpool.tile([IN_DIM, QB * 128], F32, tag="hstage")
                nc.sync.dma_start(hstage[:, : qb * 128], hT_d[:, i0 * 128:(i0 + qb) * 128])
                zstage = zpool.tile([128, QB, OUT_DIM], F32, tag="zstage")
                for j in range(qb):
                    ps = gpspool.tile([128, OUT_DIM], F32)
                    nc.tensor.matmul(ps[:], hstage[:, j * 128:(j + 1) * 128], wt[:],
                                     start=True, stop=True)
                    nc.scalar.activation(zstage[:, j, :], ps[:], AF.Copy)
                    gi = i0 + j
                    if gi * 128 < ZOWN_ROWS:
                        nc.sync.dma_start(z_own[gi * 128:(gi + 1) * 128, :], zstage[:, j, :])
                nc.sync.dma_start(z_all3[:, i0:i0 + qb, :], zstage[:, :qb, :])
            # ---------------- phase B: supers ---------------------------
            for s in range(NSUP):
                blks = list(range(s * SUPER, min((s + 1) * SUPER, NBLK)))
                M = len(blks)
                pst = [apool.tile([128, VW], F32, tag=f"ps{i}", name=f"ps{i}")
                       for i in range(M)]
                # first/last sub of each block (for start/stop flags)
                sub_seq = {b: [] for b in blks}
                for w in range(NWIN):
                    for b in blks:
                        for k in range(C[b, w] // 128):
                            sub_seq[b].append((w, k))

                for w in range(NWIN):
                    r0, rlen = region[s][w]
                    rc = rlen // 128
                    if rlen == 0:
                        continue
                    ig = spool.tile([128, RCMAX // 16], I16, tag="ig")
                    nc.sync.dma_start(ig[:, : rlen // 16], gsrc_d[:, r0 // 16:(r0 + rlen) // 16])
                    idt = spool.tile([128, RCMAX // 16], I16, tag="idt")
                    nc.sync.dma_start(idt[:, : rlen // 16], gdst_d[:, r0 // 16:(r0 + rlen) // 16])
                    slt = spool.tile([128, RCCOL], BF16, tag="slt")
                    nc.sync.dma_start(slt[:, :rc], slot_d[:, r0 // 128:(r0 + rlen) // 128])

                    zsrc = gpool.tile([128, RCCOL, OUT_DIM], F32, tag="zsrc")
                    zdst = gpool.tile([128, RCCOL, OUT_DIM], F32, tag="zdst")
                    for o2 in range(0, rlen, GMAX):
                        n2 = min(GMAX, rlen - o2)
                        nc.gpsimd.dma_gather(
                            zsrc[:, o2 // 128:(o2 + n2) // 128, :],
                            z_all[w * CHUNK_PSEUDO:(w + 1) * CHUNK_PSEUDO, :],
                            ig[:, o2 // 16:(o2 + n2) // 16], n2, n2, OUT_DIM,
                            single_packet=False)
                    for o2 in range(0, rlen, GMAX):
                        n2 = min(GMAX, rlen - o2)
                        nc.gpsimd.dma_gather(
                            zdst[:, o2 // 128:(o2 + n2) // 128, :], z_own[:],
                            idt[:, o2 // 16:(o2 + n2) // 16], n2, n2, OUT_DIM,
                            single_packet=False)

                    prod = gpool.tile([128, RCCOL, OUT_DIM], F32, tag="prod")
                    nc.vector.tensor_mul(prod[:, :rc, :], zsrc[:, :rc, :], zdst[:, :rc, :])
                    e = spool.tile([128, RCCOL], F32, tag="e")
                    nc.vector.tensor_reduce(e[:, :rc], prod[:, :rc, :], axis=AX.X, op=ALU.add)
                    mx = spool.tile([128, RCCOL], F32, tag="mx")
                    mn = spool.tile([128, RCCOL], F32, tag="mn")
                    nc.vector.tensor_scalar(mx[:, :rc], e[:, :rc], 0.0, -48.0, op0=ALU.max, op1=ALU.add)
                    nc.vector.tensor_scalar_min(mn[:, :rc], e[:, :rc], 0.0)
                    lr = spool.tile([128, RCCOL], F32, tag="lr")
                    nc.vector.scalar_tensor_tensor(lr[:, :rc], in0=mn[:, :rc], scalar=SLOPE,
                                                   in1=mx[:, :rc], op0=ALU.mult, op1=ALU.add)
                    ex = spool.tile([128, RCCOL], F32, tag="ex")
                    nc.scalar.activation(ex[:, :rc], lr[:, :rc], AF.Exp)

                    st = gpool.tile([128, RCCOL, 128], BF16, tag="st")
                    if DEBUG:
                        nc.sync.dma_start(e_d[:, r0 // 128:(r0 + rlen) // 128], e[:, :rc])

                    nc.vector.tensor_tensor(
                        st[:, :rc, :],
                        slt[:, :rc, None].broadcast_to((128, rc, 128)),
                        it_t[:, None, :].broadcast_to((128, rc, 128)),
                        op=ALU.is_equal)
                    if DEBUG and s == 0 and w == 0:
                        sf = gpool.tile([128, RCCOL, 128], F32, tag="sf")
                        nc.vector.tensor_copy(sf[:, :rc, :], st[:, :rc, :])
                        nc.sync.dma_start(s_d[:, :rc, :], sf[:, :rc, :])
                    vals = gpool.tile([128, RCCOL, VW + 1], BF16, tag="vals")
                    exb = ex[:, :rc, None].broadcast_to((128, rc, OUT_DIM))
                    nc.vector.tensor_mul(vals[:, :rc, 0:OUT_DIM], zsrc[:, :rc, :], exb)
                    nc.vector.tensor_copy(vals[:, :rc, OUT_DIM], ex[:, :rc])

                    for bi, b in enumerate(blks):
                        base = (chunk_off[b, w] - r0) // 128
                        for k in range(C[b, w] // 128):
                            first = sub_seq[b][0] == (w, k)
                            last = sub_seq[b][-1] == (w, k)
                            nc.tensor.matmul(pst[bi][:], st[:, base + k, :],
                                             vals[:, base + k, 0:VW],
                                             start=first, stop=last)

                # ---------------- normalize + elu + write ----------------
                stg = fpool.tile([128, SUPER, VW], F32, tag="stg")
                for bi in range(M):
                    nc.scalar.activation(stg[:, bi, :], pst[bi][:], AF.Copy)
                den = fpool.tile([128, SUPER], F32, tag="den")
                nc.vector.tensor_scalar_add(den[:, :M], stg[:, :M, OUT_DIM], 1e-37)
                rcp = fpool.tile([128, SUPER], F32, tag="rcp")
                nc.vector.reciprocal(rcp[:, :M], den[:, :M])
                o64 = fpool.tile([128, SUPER, OUT_DIM], F32, tag="o64")
                nc.vector.tensor_mul(o64[:, :M, :], stg[:, :M, 0:OUT_DIM],
                                     rcp[:, :M, None].broadcast_to((128, M, OUT_DIM)))
                mn2 = fpool.tile([128, SUPER, OUT_DIM], F32, tag="mn2")
                nc.vector.tensor_scalar_min(mn2[:, :M, :], o64[:, :M, :], 0.0)
                emn = fpool.tile([128, SUPER, OUT_DIM], F32, tag="emn")
                nc.scalar.activation(emn[:, :M, :], mn2[:, :M, :], AF.Exp)
                mx2 = fpool.tile([128, SUPER, OUT_DIM], F32, tag="mx2")
                nc.vector.tensor_scalar_max(mx2[:, :M, :], o64[:, :M, :], 0.0)
                res = fpool.tile([128, SUPER, OUT_DIM], F32, tag="res")
                nc.vector.scalar_tensor_tensor(res[:, :M, :], in0=emn[:, :M, :], scalar=-1.0,
                                               in1=mx2[:, :M, :], op0=ALU.add, op1=ALU.add)
                if DEBUG:
                    nc.sync.dma_start(astg_d[s, :, :M, :], stg[:, :M, :])
                out3 = out_d[:].rearrange("(b p) d -> p b d", p=128)
                nc.sync.dma_start(out3[:, s * SUPER:s * SUPER + M, :], res[:, :M, :])

    nc.finalize()
    return nc, in_maps


def kernel(h, W, src, dst):
    global LAST_RESULTS
    nc, in_maps = _build(h, W, src, dst)
    results = run_bass_kernel_spmd(
        nc, in_maps, core_ids=list(range(NC)),
        trace=bool(int(os.environ.get("GAT_TRACE", "0"))),
    )
    LAST_RESULTS = results
    out = np.concatenate(
        [results.results[c]["out"][:NPC] for c in range(NC)], axis=0)
    return out.astype(np.float32)


# revision 14
# speedup vs baseline: 1.0303x; 1.0303x over previous
"""GAT layer (segment-softmax message passing) on 8 Trainium2 NeuronCores.

Strategy (per core c of NC=8, SPMD single program, per-core input maps):
  - Nodes sharded by destination: core c owns dst rows [c*NPC, (c+1)*NPC).
  - Each core computes the full z = h @ W^T itself (hT pre-rolled so its own
    nodes are rows [0, NPC)), storing z twice in DRAM:
      z_all : partition-major pseudo-row layout (gather source for z_src)
      z_own : row-major first ZOWN_ROWS rows (gather source for z_dst)
  - Edges are bucketed by (dst block of 128 nodes, src int16-gather window).
    Blocks are grouped into supers of 8. Slot layout per super: for each of
    the 4 windows, the 8 blocks' chunks concatenated (each chunk padded to
    x128, capacity = max over cores, so the layout is core-invariant).
  - Per (super, window) region: one chunked dma_gather of z_src from the
    window of z_all and one of z_dst from z_own; DVE dot + leaky-relu;
    ScalarE exp (no max subtraction: softmax is shift-invariant and fp32
    holds exp(~45)); vals = [ex * z_src, ex] in bf16; a one-hot S matrix
    [128 edges x 128 slots] built on DVE from host slotids (is_equal vs an
    iota row; padding slots get slotid=-1 so their column is all-zero).
  - Aggregation: per 128-edge sub-tile, TensorE matmul S^T @ vals
    accumulates into the dst block's PSUM tile [128, 65] (fp32 accumulate,
    start/stop over the block's subs). No scatter-add, no dst-row
    duplication concerns, no virtual nodes.
  - After a super's 4 regions: PSUM -> SBUF, normalize by the ex column,
    ELU, and one contiguous DMA write of the 8 blocks' output rows.

The host does only sharding/layout work: bucketing, padding, int16/bf16
index tensors, and the h^T roll. All FLOPs happen on device.
"""

import os
import sys

sys.path.insert(0, "/opt/trn_rl_repo")

import ml_dtypes
import numpy as np

import concourse.bacc as bacc
import concourse.mybir as mybir
import concourse.tile as tile
from concourse.bass_utils import run_bass_kernel_spmd

F32 = mybir.dt.float32
BF16 = mybir.dt.bfloat16
I16 = mybir.dt.int16
AF = mybir.ActivationFunctionType
ALU = mybir.AluOpType
AX = mybir.AxisListType

LAST_RESULTS = None  # test harness reads exec_time_ns from here

N = 100000
IN_DIM = 128
OUT_DIM = 64
NC = 8
NPC = N // NC  # 12500
SLOPE = 0.2

NT_G = (N + 127) // 128  # 782 GEMM row tiles
NROWS = NT_G * 128  # 100096 padded node rows
NWIN = 4
CHUNK_PSEUDO = NROWS // NWIN  # 25024 pseudo rows per int16 gather window
ZOWN_ROWS = ((NPC + 127) // 128) * 128  # 12544
NBLK = ZOWN_ROWS // 128  # 98 dst blocks per core
SUPER = 4  # dst blocks per super-tile
NSUP = (NBLK + SUPER - 1) // SUPER  # 13
GMAX = 1024  # max idxs per SWDGE gather call (single-packet limit)
VW = 65  # vals width: 64 features + ex column


def _wrap16(a):
    """[n] int -> [128, n//16] int16 (element i at [i%16, i//16], tiled x8)."""
    w = a.astype(np.int16).reshape(-1, 16).T.copy()
    return np.tile(w, (8, 1))


def _tile128(a, dtype):
    """[n] -> [128, n//128] (element j at [j%128, j//128])."""
    return np.ascontiguousarray(a.reshape(-1, 128).T).astype(dtype)


def _plan(h, W, src, dst):
    h = np.asarray(h, np.float32)
    W = np.asarray(W, np.float32)
    src = np.asarray(src).astype(np.int64)
    dst = np.asarray(dst).astype(np.int64)

    # ---- per-core edge bucketing by (dst block, src window) -------------
    per_core = []
    counts = np.zeros((NC, NBLK, NWIN), np.int64)
    for c in range(NC):
        sel = np.flatnonzero(dst // NPC == c)
        d_l = (dst[sel] - c * NPC).astype(np.int32)
        s_r = ((src[sel] - c * NPC) % N).astype(np.int64)
        pseudo = ((s_r % 128) * NT_G + s_r // 128).astype(np.int32)
        w = pseudo // CHUNK_PSEUDO
        blk = d_l >> 7
        key = (blk.astype(np.int64) * NWIN + w) * E_KEY + pseudo
        order = np.argsort(key, kind="stable")
        d_l, pseudo, w, blk = d_l[order], pseudo[order], w[order], blk[order]
        np.add.at(counts[c], (blk, w), 1)
        per_core.append((d_l, pseudo, w, blk))

    C = counts.max(axis=0)  # [NBLK, NWIN]
    C = ((C + 127) // 128) * 128
    C[:, 0] = np.maximum(C[:, 0], 128)  # every block gets >=1 sub

    # ---- static slot layout (identical across cores) --------------------
    chunk_off = np.zeros((NBLK, NWIN), np.int64)
    region = []  # per super: list over w of (region_off, region_len)
    off = 0
    for s in range(NSUP):
        blks = range(s * SUPER, min((s + 1) * SUPER, NBLK))
        regs = []
        for w in range(NWIN):
            r0 = off
            for b in blks:
                chunk_off[b, w] = off
                off += C[b, w]
            regs.append((r0, off - r0))
        region.append(regs)
    TOT = off
    RCMAX = max(l for regs in region for _, l in regs)

    # ---- per-core index tensors -----------------------------------------
    hT = np.ascontiguousarray(h.T)
    wT = np.ascontiguousarray(W.T)
    iota = np.broadcast_to(np.arange(128, dtype=np.float32), (128, 128))
    in_maps = []
    for c in range(NC):
        d_l, pseudo, w, blk = per_core[c]
        gsrc = np.zeros(TOT, np.int32)
        gdst = np.zeros(TOT, np.int32)
        slotid = np.full(TOT, -1.0, np.float32)
        # edges are sorted by (blk, w, pseudo); chunks are contiguous runs
        cnt = counts[c]
        pos = 0
        for b in range(NBLK):
            for ww in range(NWIN):
                k = cnt[b, ww]
                if k == 0:
                    continue
                o = chunk_off[b, ww]
                sl = slice(pos, pos + k)
                gsrc[o:o + k] = pseudo[sl] - ww * CHUNK_PSEUDO
                gdst[o:o + k] = d_l[sl]
                slotid[o:o + k] = (d_l[sl] & 127).astype(np.float32)
                pos += k
        assert pos == len(d_l)
        hp = np.zeros((IN_DIM, NROWS), np.float32)
        hp[:, :N] = np.roll(hT, -c * NPC, axis=1)
        in_maps.append({
            "hT": hp,
            "wT": wT,
            "iota": iota.astype(ml_dtypes.bfloat16),
            "gsrc_idx": _wrap16(gsrc),
            "gdst_idx": _wrap16(gdst),
            "slotid": _tile128(slotid, ml_dtypes.bfloat16),
        })
    return in_maps, C, chunk_off, region, TOT, RCMAX


E_KEY = 1 << 18  # > CHUNK_PSEUDO, for host-side lexsort key


def _build(h, W, src, dst):
    in_maps, C, chunk_off, region, TOT, RCMAX = _plan(h, W, src, dst)
    RCCOL = RCMAX // 128

    nc = bacc.Bacc(None, target_bir_lowering=False, debug=False)
    hT_d = nc.declare_dram_parameter("hT", [IN_DIM, NROWS], F32, isOutput=False)
    wT_d = nc.declare_dram_parameter("wT", [IN_DIM, OUT_DIM], F32, isOutput=False)
    iota_d = nc.declare_dram_parameter("iota", [128, 128], BF16, isOutput=False)
    gsrc_d = nc.declare_dram_parameter("gsrc_idx", [128, TOT // 16], I16, isOutput=False)
    gdst_d = nc.declare_dram_parameter("gdst_idx", [128, TOT // 16], I16, isOutput=False)
    slot_d = nc.declare_dram_parameter("slotid", [128, TOT // 128], BF16, isOutput=False)
    out_d = nc.declare_dram_parameter("out", [ZOWN_ROWS, OUT_DIM], F32, isOutput=True)
    DEBUG = bool(int(os.environ.get("GAT_DEBUG", "0")))
    if DEBUG:
        astg_d = nc.declare_dram_parameter("stg_dump", [NSUP, 128, SUPER, VW], F32, isOutput=True)
        e_d = nc.declare_dram_parameter("e_dump", [128, TOT // 128], F32, isOutput=True)
        s_d = nc.declare_dram_parameter("s_dump", [128, RCMAX // 128, 128], F32, isOutput=True)

    z_all = nc.dram_tensor("z_all", [NROWS, OUT_DIM], F32)  # pseudo rows
    z_own = nc.dram_tensor("z_own", [ZOWN_ROWS, OUT_DIM], F32)

    QB = 8
    with tile.TileContext(nc) as tc:
        with tc.tile_pool(name="w", bufs=1) as wpool, \
             tc.tile_pool(name="gps", bufs=4, space="PSUM") as gpspool, \
             tc.tile_pool(name="agg", bufs=1, space="PSUM") as apool, \
             tc.tile_pool(name="hst", bufs=3) as hpool, \
             tc.tile_pool(name="zst", bufs=3) as zpool, \
             tc.tile_pool(name="gat", bufs=2) as gpool, \
             tc.tile_pool(name="sm", bufs=2) as spool, \
             tc.tile_pool(name="fin", bufs=2) as fpool:

            # ---------------- phase A: z = h @ W^T ----------------------
            wt = wpool.tile([IN_DIM, OUT_DIM], F32)
            nc.sync.dma_start(wt[:], wT_d[:])
            it_t = wpool.tile([128, 128], BF16)
            nc.sync.dma_start(it_t[:], iota_d[:])
            z_all3 = z_all[:].rearrange("(p i) d -> p i d", p=128)
            for i0 in range(0, NT_G, QB):
                qb = min(QB, NT_G - i0)
                hstage = hpool.tile([IN_DIM, QB * 128], F32, tag="hstage")
                nc.sync.dma_start(hstage[:, : qb * 128], hT_d[:, i0 * 128:(i0 + qb) * 128])
                zstage = zpool.tile([128, QB, OUT_DIM], F32, tag="zstage")
                for j in range(qb):
                    ps = gpspool.tile([128, OUT_DIM], F32)
                    nc.tensor.matmul(ps[:], hstage[:, j * 128:(j + 1) * 128], wt[:],
                                     start=True, stop=True)
                    nc.scalar.activation(zstage[:, j, :], ps[:], AF.Copy)
                    gi = i0 + j
                    if gi * 128 < ZOWN_ROWS:
                        nc.sync.dma_start(z_own[gi * 128:(gi + 1) * 128, :], zstage[:, j, :])
                nc.sync.dma_start(z_all3[:, i0:i0 + qb, :], zstage[:, :qb, :])
            # ---------------- phase B: supers ---------------------------
            for s in range(NSUP):
                blks = list(range(s * SUPER, min((s + 1) * SUPER, NBLK)))
                M = len(blks)
                pst = [apool.tile([128, VW], F32, tag=f"ps{i}", name=f"ps{i}")
                       for i in range(M)]
                # first/last sub of each block (for start/stop flags)
                sub_seq = {b: [] for b in blks}
                for w in range(NWIN):
                    for b in blks:
                        for k in range(C[b, w] // 128):
                            sub_seq[b].append((w, k))

                for w in range(NWIN):
                    r0, rlen = region[s][w]
                    rc = rlen // 128
                    if rlen == 0:
                        continue
                    ig = spool.tile([128, RCMAX // 16], I16, tag="ig")
                    nc.sync.dma_start(ig[:, : rlen // 16], gsrc_d[:, r0 // 16:(r0 + rlen) // 16])
                    idt = spool.tile([128, RCMAX // 16], I16, tag="idt")
                    nc.sync.dma_start(idt[:, : rlen // 16], gdst_d[:, r0 // 16:(r0 + rlen) // 16])
                    slt = spool.tile([128, RCCOL], BF16, tag="slt")
                    nc.sync.dma_start(slt[:, :rc], slot_d[:, r0 // 128:(r0 + rlen) // 128])

                    zsrc = gpool.tile([128, RCCOL, OUT_DIM], F32, tag="zsrc")
                    zdst = gpool.tile([128, RCCOL, OUT_DIM], F32, tag="zdst")
                    for o2 in range(0, rlen, GMAX):
                        n2 = min(GMAX, rlen - o2)
                        nc.gpsimd.dma_gather(
                            zsrc[:, o2 // 128:(o2 + n2) // 128, :],
                            z_all[w * CHUNK_PSEUDO:(w + 1) * CHUNK_PSEUDO, :],
                            ig[:, o2 // 16:(o2 + n2) // 16], n2, n2, OUT_DIM,
                            single_packet=True)
                    for o2 in range(0, rlen, GMAX):
                        n2 = min(GMAX, rlen - o2)
                        nc.gpsimd.dma_gather(
                            zdst[:, o2 // 128:(o2 + n2) // 128, :], z_own[:],
                            idt[:, o2 // 16:(o2 + n2) // 16], n2, n2, OUT_DIM,
                            single_packet=True)

                    prod = gpool.tile([128, RCCOL, OUT_DIM], F32, tag="prod")
                    nc.vector.tensor_mul(prod[:, :rc, :], zsrc[:, :rc, :], zdst[:, :rc, :])
                    e = spool.tile([128, RCCOL], F32, tag="e")
                    nc.vector.tensor_reduce(e[:, :rc], prod[:, :rc, :], axis=AX.X, op=ALU.add)
                    mx = spool.tile([128, RCCOL], F32, tag="mx")
                    mn = spool.tile([128, RCCOL], F32, tag="mn")
                    nc.vector.tensor_scalar(mx[:, :rc], e[:, :rc], 0.0, -48.0, op0=ALU.max, op1=ALU.add)
                    nc.vector.tensor_scalar_min(mn[:, :rc], e[:, :rc], 0.0)
                    lr = spool.tile([128, RCCOL], F32, tag="lr")
                    nc.vector.scalar_tensor_tensor(lr[:, :rc], in0=mn[:, :rc], scalar=SLOPE,
                                                   in1=mx[:, :rc], op0=ALU.mult, op1=ALU.add)
                    ex = spool.tile([128, RCCOL], F32, tag="ex")
                    nc.scalar.activation(ex[:, :rc], lr[:, :rc], AF.Exp)

                    st = gpool.tile([128, RCCOL, 128], BF16, tag="st")
                    if DEBUG:
                        nc.sync.dma_start(e_d[:, r0 // 128:(r0 + rlen) // 128], e[:, :rc])

                    nc.vector.tensor_tensor(
                        st[:, :rc, :],
                        slt[:, :rc, None].broadcast_to((128, rc, 128)),
                        it_t[:, None, :].broadcast_to((128, rc, 128)),
                        op=ALU.is_equal)
                    if DEBUG and s == 0 and w == 0:
                        sf = gpool.tile([128, RCCOL, 128], F32, tag="sf")
                        nc.vector.tensor_copy(sf[:, :rc, :], st[:, :rc, :])
                        nc.sync.dma_start(s_d[:, :rc, :], sf[:, :rc, :])
                    vals = gpool.tile([128, RCCOL, VW + 1], BF16, tag="vals")
                    exb = ex[:, :rc, None].broadcast_to((128, rc, OUT_DIM))
                    nc.vector.tensor_mul(vals[:, :rc, 0:OUT_DIM], zsrc[:, :rc, :], exb)
                    nc.vector.tensor_copy(vals[:, :rc, OUT_DIM], ex[:, :rc])

                    for bi, b in enumerate(blks):
                        base = (chunk_off[b, w] - r0) // 128
                        for k in range(C[b, w] // 128):
                            first = sub_seq[b][0] == (w, k)
                            last = sub_seq[b][-1] == (w, k)
                            nc.tensor.matmul(pst[bi][:], st[:, base + k, :],
                                             vals[:, base + k, 0:VW],
                                             start=first, stop=last)

                # ---------------- normalize + elu + write ----------------
                stg = fpool.tile([128, SUPER, VW], F32, tag="stg")
                for bi in range(M):
                    nc.scalar.activation(stg[:, bi, :], pst[bi][:], AF.Copy)
                den = fpool.tile([128, SUPER], F32, tag="den")
                nc.vector.tensor_scalar_add(den[:, :M], stg[:, :M, OUT_DIM], 1e-37)
                rcp = fpool.tile([128, SUPER], F32, tag="rcp")
                nc.vector.reciprocal(rcp[:, :M], den[:, :M])
                o64 = fpool.tile([128, SUPER, OUT_DIM], F32, tag="o64")
                nc.vector.tensor_mul(o64[:, :M, :], stg[:, :M, 0:OUT_DIM],
                                     rcp[:, :M, None].broadcast_to((128, M, OUT_DIM)))
                mn2 = fpool.tile([128, SUPER, OUT_DIM], F32, tag="mn2")
                nc.vector.tensor_scalar_min(mn2[:, :M, :], o64[:, :M, :], 0.0)
                emn = fpool.tile([128, SUPER, OUT_DIM], F32, tag="emn")
                nc.scalar.activation(emn[:, :M, :], mn2[:, :M, :], AF.Exp)
                mx2 = fpool.tile([128, SUPER, OUT_DIM], F32, tag="mx2")
                nc.vector.tensor_scalar_max(mx2[:, :M, :], o64[:, :M, :], 0.0)
                res = fpool.tile([128, SUPER, OUT_DIM], F32, tag="res")
                nc.vector.scalar_tensor_tensor(res[:, :M, :], in0=emn[:, :M, :], scalar=-1.0,
                                               in1=mx2[:, :M, :], op0=ALU.add, op1=ALU.add)
                if DEBUG:
                    nc.sync.dma_start(astg_d[s, :, :M, :], stg[:, :M, :])
                out3 = out_d[:].rearrange("(b p) d -> p b d", p=128)
                nc.sync.dma_start(out3[:, s * SUPER:s * SUPER + M, :], res[:, :M, :])

    nc.finalize()
    return nc, in_maps


def kernel(h, W, src, dst):
    global LAST_RESULTS
    nc, in_maps = _build(h, W, src, dst)
    results = run_bass_kernel_spmd(
        nc, in_maps, core_ids=list(range(NC)),
        trace=bool(int(os.environ.get("GAT_TRACE", "0"))),
    )
    LAST_RESULTS = results
    out = np.concatenate(
        [results.results[c]["out"][:NPC] for c in range(NC)], axis=0)
    return out.astype(np.float32)
